# revision 1
# baseline (speedup 1.0000x reference)
import math

import numpy as np

# GCNII layer constants (match the reference problem definition).
N = 100000
D = 32
ALPHA = 0.1
THETA = 0.5
LAYER = 8
BETA = math.log(THETA / (LAYER + 1) + 1.0)


def _compute_numpy(x, x_0, edge_index, weight1):
    src = np.asarray(edge_index[0], dtype=np.int64)
    dst = np.asarray(edge_index[1], dtype=np.int64)
    x = np.asarray(x, dtype=np.float32)
    x_0 = np.asarray(x_0, dtype=np.float32)
    weight1 = np.asarray(weight1, dtype=np.float32)

    # agg[i] = sum_{e: dst[e]==i} x[src[e]]  — one bincount per feature
    # column is much faster than np.add.at on a [E, D] gather.
    gathered = x[src]  # [E, D]
    agg = np.empty((N, D), dtype=np.float32)
    for d in range(D):
        agg[:, d] = np.bincount(dst, weights=gathered[:, d], minlength=N)

    out = (1.0 - ALPHA) * agg + ALPHA * x_0
    out = (1.0 - BETA) * out + BETA * (out @ weight1)
    return out.astype(np.float32)


def _compute_jax_neuron_local(x, x_0, edge_index, weight1):
    """Edges pre-bucketed by destination shard on host: each core only
    receives edges whose dst lands in its node range, so the scatter-add
    is local and no cross-core reduction of partials is needed."""
    import jax
    import jax.numpy as jnp
    from jax.sharding import Mesh, PartitionSpec as P
    from jax.experimental.shard_map import shard_map

    devs = jax.devices()
    n_cores = 8
    if len(devs) < n_cores:
        raise RuntimeError("need 8 cores")
    mesh = Mesh(np.array(devs[:n_cores]), ("i",))

    n_loc = N // n_cores  # 12500
    src = np.asarray(edge_index[0], dtype=np.int32)
    dst = np.asarray(edge_index[1], dtype=np.int32)
    bucket = dst // n_loc
    order = np.argsort(bucket, kind="stable")
    src_s, dst_s = src[order], dst[order]
    counts = np.bincount(bucket, minlength=n_cores)
    cap = int(counts.max())
    # Pad each bucket to `cap`; pad edges get local dst == n_loc, which is
    # out of range for segment_sum(num_segments=n_loc) and is dropped.
    src_p = np.zeros((n_cores, cap), dtype=np.int32)
    dst_p = np.full((n_cores, cap), n_loc, dtype=np.int32)
    off = 0
    for c in range(n_cores):
        cnt = int(counts[c])
        src_p[c, :cnt] = src_s[off : off + cnt]
        dst_p[c, :cnt] = dst_s[off : off + cnt] - c * n_loc
        off += cnt

    xj = jnp.asarray(x, dtype=jnp.float32)
    x0j = jnp.asarray(x_0, dtype=jnp.float32)
    wj = jnp.asarray(weight1, dtype=jnp.float32)
    srcj = jnp.asarray(src_p)
    dstj = jnp.asarray(dst_p)

    def shard_fn(src_l, dst_l, x_full, x0_l, w):
        gathered = x_full[src_l[0]]  # [cap, D]
        agg_l = jax.ops.segment_sum(gathered, dst_l[0], num_segments=n_loc)
        out = (1.0 - ALPHA) * agg_l + ALPHA * x0_l
        out = (1.0 - BETA) * out + BETA * (out @ w)
        return out

    fn = jax.jit(
        shard_map(
            shard_fn,
            mesh=mesh,
            in_specs=(P("i"), P("i"), P(), P("i"), P()),
            out_specs=P("i"),
        )
    )
    out = fn(srcj, dstj, xj, x0j, wj)
    return np.asarray(jax.device_get(out), dtype=np.float32)


def _compute_jax_neuron(x, x_0, edge_index, weight1):
    """Run the layer on the Trainium cores via PJRT, edges sharded 8-way.

    Each core takes a 1/8 slice of the edge list, gathers source rows from
    a replicated x, and scatter-adds into a full-size [N, D] partial;
    partials are summed across cores (psum), then the dense GCNII
    combination runs replicated.
    """
    import jax
    import jax.numpy as jnp
    from jax.sharding import Mesh, PartitionSpec as P
    from jax.experimental.shard_map import shard_map

    devs = jax.devices()
    n_cores = 8
    if len(devs) < n_cores:
        raise RuntimeError("need 8 cores")
    mesh = Mesh(np.array(devs[:n_cores]), ("i",))

    E = edge_index.shape[1]
    assert E % n_cores == 0

    src = jnp.asarray(np.asarray(edge_index[0], dtype=np.int32))
    dst = jnp.asarray(np.asarray(edge_index[1], dtype=np.int32))
    xj = jnp.asarray(x, dtype=jnp.float32)
    x0j = jnp.asarray(x_0, dtype=jnp.float32)
    wj = jnp.asarray(weight1, dtype=jnp.float32)

    def shard_fn(src_l, dst_l, x_full, x0_l, w):
        gathered = x_full[src_l]  # [E/8, D]
        partial = jax.ops.segment_sum(gathered, dst_l, num_segments=N)
        agg = jax.lax.psum(partial, "i")  # [N, D] replicated
        n_loc = x0_l.shape[0]
        idx = jax.lax.axis_index("i") * n_loc
        agg_l = jax.lax.dynamic_slice_in_dim(agg, idx, n_loc, axis=0)
        out = (1.0 - ALPHA) * agg_l + ALPHA * x0_l
        out = (1.0 - BETA) * out + BETA * (out @ w)
        return out

    fn = jax.jit(
        shard_map(
            shard_fn,
            mesh=mesh,
            in_specs=(P("i"), P("i"), P(), P("i"), P()),
            out_specs=P("i"),
        )
    )
    out = fn(src, dst, xj, x0j, wj)
    return np.asarray(jax.device_get(out), dtype=np.float32)


def kernel(x, x_0, edge_index, weight1):
    try:
        return _compute_jax_neuron_local(x, x_0, edge_index, weight1)
    except Exception:
        pass
    try:
        return _compute_jax_neuron(x, x_0, edge_index, weight1)
    except Exception:
        return _compute_numpy(x, x_0, edge_index, weight1)



# revision 7
# speedup vs baseline: 1.8153x; 1.8153x over previous
import math

import numpy as np

# GCNII layer constants (match the reference problem definition).
N = 100000
D = 32
NCORES = 8
NLOC = N // NCORES  # 12500
ALPHA = 0.1
THETA = 0.5
LAYER = 8
BETA = math.log(THETA / (LAYER + 1) + 1.0)

# Device tiling constants.
WIN = 128            # nodes per window (= one-hot matmul width)
NWIN = 98            # windows per core (97*128 + 84 = 12500)
TPW = 17             # tiles (of 128 edge slots) per window
CAP = TPW * 128      # 2176 edge slots per window
CHW = 7              # windows per gather chunk
NCH = NWIN // CHW    # 14 chunks per core
TILES_CH = CHW * TPW  # 119 tiles per chunk
NODES_PAD = NWIN * WIN  # 12544

_BASS_PROG = None  # cached (nc, names) across calls in one process


def _np_bf16():
    import ml_dtypes

    return ml_dtypes.bfloat16


def _preprocess(x, x_0, edge_index, weight1):
    """Sort edges by destination, bucket into per-core / per-window padded
    slots, and build the device-side index/feature layouts."""
    bf16 = _np_bf16()
    src = np.ascontiguousarray(edge_index[0], dtype=np.int32)
    dst = np.ascontiguousarray(edge_index[1], dtype=np.int32)
    E = src.shape[0]

    order = np.argsort(dst, kind="stable")
    ds = dst[order]
    ss = src[order]

    core = ds // NLOC
    loc = ds - core * NLOC
    wl = loc // WIN
    gwin = core * NWIN + wl

    counts = np.bincount(gwin, minlength=NCORES * NWIN)
    starts = np.concatenate([[0], np.cumsum(counts)[:-1]])
    rank = np.arange(E, dtype=np.int64) - starts[gwin]
    keep = rank < CAP

    t_in_core = wl * TPW + rank // WIN
    lane = rank % WIN
    ch = t_in_core // TILES_CH
    tau = t_in_core - ch * TILES_CH
    pos = ((core * NCH + ch) * 128 + lane) * TILES_CH + tau

    nslots = NCORES * NCH * 128 * TILES_CH
    src_dev = np.zeros(nslots, dtype=np.int32)
    src_dev[pos[keep]] = ss[keep]
    dr_dev = np.full(nslots, -1.0, dtype=np.float32)
    dr_dev[pos[keep]] = (loc - wl * WIN)[keep]

    src_dev = src_dev.reshape(NCORES, NCH, 128, TILES_CH)
    dr_dev = dr_dev.astype(bf16).reshape(NCORES, NCH, 128, TILES_CH)

    x_bf = np.ascontiguousarray(x, dtype=np.float32).astype(bf16)

    # x0 feature-major per core, padded to NODES_PAD nodes.
    x0T = np.ascontiguousarray(np.asarray(x_0, dtype=np.float32).T)  # [32, N]
    x0fm = np.zeros((NCORES, D, NODES_PAD), dtype=np.float32)
    for c in range(NCORES):
        x0fm[c, :, :NLOC] = x0T[:, c * NLOC : (c + 1) * NLOC]
    x0fm = x0fm.astype(bf16)

    w_f32 = np.ascontiguousarray(weight1, dtype=np.float32)

    spill = None
    if not np.all(keep):
        sp = ~keep
        spill = (ss[sp], ds[sp])
    return x_bf, x0fm, src_dev, dr_dev, w_f32, spill


def _build_bass_program():
    global _BASS_PROG
    if _BASS_PROG is not None:
        return _BASS_PROG

    import concourse.bass as bass
    import concourse.mybir as mybir
    from concourse.bass import IndirectOffsetOnAxis
    from concourse.masks import make_identity
    from concourse.tile import TileContext

    dt = mybir.dt
    op = mybir.AluOpType

    nc = bass.Bass()
    x_d = nc.declare_dram_parameter("x", [N, D], dt.bfloat16, False)
    x0_d = nc.declare_dram_parameter("x0", [D, NODES_PAD], dt.bfloat16, False)
    src_d = nc.declare_dram_parameter("src", [NCH, 128, TILES_CH], dt.int32, False)
    dr_d = nc.declare_dram_parameter("dr", [NCH, 128, TILES_CH], dt.bfloat16, False)
    w_d = nc.declare_dram_parameter("w", [D, D], dt.float32, False)
    out_d = nc.declare_dram_parameter("out", [128, NWIN * D], dt.float32, True)

    with TileContext(nc) as tc:
        with (
            tc.tile_pool(name="const", bufs=1) as constp,
            tc.tile_pool(name="idx", bufs=2) as idxp,
            tc.tile_pool(name="drt", bufs=2) as drp,
            tc.tile_pool(name="gat", bufs=2) as gp,
            tc.tile_pool(name="sel", bufs=3) as sp_,
            tc.tile_pool(name="agg", bufs=3) as abp,
            tc.tile_pool(name="pagg", bufs=3, space="PSUM") as pap,
            tc.tile_pool(name="pout", bufs=2, space="PSUM") as p2p,
        ):
            # Constants: iota [128, TPW, 128] (value = free index within
            # window), identity, and the two GCNII combination matrices
            # Ma = 0.9*((1-b)I + b*W), Mb = Ma/9.
            iota_i = constp.tile([128, TPW, WIN], dt.int16)
            nc.gpsimd.iota(
                iota_i[:, :, :], pattern=[[0, TPW], [1, WIN]], base=0,
                channel_multiplier=0,
            )
            iota_bf = constp.tile([128, TPW, WIN], dt.bfloat16)
            nc.vector.tensor_copy(iota_bf[:, :, :], iota_i[:, :, :])

            w_sb = constp.tile([D, D], dt.float32)
            nc.sync.dma_start(out=w_sb[:, :], in_=w_d[:, :])
            eye = constp.tile([D, D], dt.float32)
            make_identity(nc, eye[:, :])
            wa = constp.tile([D, D], dt.float32)
            nc.vector.tensor_scalar_mul(wa[:, :], w_sb[:, :], 0.9 * BETA)
            wb = constp.tile([D, D], dt.float32)
            nc.vector.tensor_scalar_mul(wb[:, :], w_sb[:, :], 0.1 * BETA)
            ma = constp.tile([D, D], dt.bfloat16)
            nc.vector.scalar_tensor_tensor(
                out=ma[:, :], in0=eye[:, :], scalar=0.9 * (1.0 - BETA),
                in1=wa[:, :], op0=op.mult, op1=op.add,
            )
            mb = constp.tile([D, D], dt.bfloat16)
            nc.vector.scalar_tensor_tensor(
                out=mb[:, :], in0=eye[:, :], scalar=0.1 * (1.0 - BETA),
                in1=wb[:, :], op0=op.mult, op1=op.add,
            )

            x0_sb = constp.tile([D, NODES_PAD], dt.bfloat16)
            nc.sync.dma_start(out=x0_sb[:, :], in_=x0_d[:, :])

            staging = constp.tile([128, NWIN * D], dt.float32)

            pa = None
            p2 = None
            for chi in range(NCH):
                idx_t = idxp.tile([128, TILES_CH], dt.int32)
                nc.sync.dma_start(out=idx_t[:, :], in_=src_d[chi, :, :])
                dr_t = drp.tile([128, TILES_CH], dt.bfloat16)
                nc.sync.dma_start(out=dr_t[:, :], in_=dr_d[chi, :, :])

                g_t = gp.tile([128, TILES_CH, D], dt.bfloat16)
                nc.gpsimd.indirect_dma_start(
                    out=g_t[:, :, :],
                    out_offset=None,
                    in_=x_d[:, :],
                    in_offset=IndirectOffsetOnAxis(ap=idx_t[:, :], axis=0),
                )

                for m in range(CHW):
                    w = chi * CHW + m
                    s_t = sp_.tile([128, TPW, WIN], dt.bfloat16)
                    nc.vector.tensor_tensor(
                        out=s_t[:, :, :],
                        in0=iota_bf[:, :, :],
                        in1=dr_t[:, m * TPW : (m + 1) * TPW].to_broadcast(
                            [128, TPW, WIN]
                        ),
                        op=op.is_equal,
                    )
                    if w % 4 == 0:
                        pa = pap.tile([D, 4 * WIN], dt.float32)
                    for t in range(TPW):
                        nc.tensor.matmul(
                            out=pa[:, (w % 4) * WIN : (w % 4 + 1) * WIN],
                            lhsT=g_t[:, m * TPW + t, :],
                            rhs=s_t[:, t, :],
                            start=(t == 0),
                            stop=(t == TPW - 1),
                        )
                    if w % 4 == 3 or w == NWIN - 1:
                        g0 = (w // 4) * 4
                        ncols = (w - g0 + 1) * WIN
                        ab = abp.tile([D, 4 * WIN], dt.bfloat16)
                        nc.any.tensor_copy(ab[:, :ncols], pa[:, :ncols])
                        for k in range(g0, w + 1):
                            if k % 16 == 0:
                                p2 = p2p.tile([128, 16 * D], dt.float32)
                            c0 = (k % 16) * D
                            nc.tensor.matmul(
                                out=p2[:, c0 : c0 + D],
                                lhsT=ab[:, (k - g0) * WIN : (k - g0 + 1) * WIN],
                                rhs=ma[:, :],
                                start=True,
                                stop=False,
                            )
                            nc.tensor.matmul(
                                out=p2[:, c0 : c0 + D],
                                lhsT=x0_sb[:, k * WIN : (k + 1) * WIN],
                                rhs=mb[:, :],
                                start=False,
                                stop=True,
                            )
                            if k % 16 == 15 or k == NWIN - 1:
                                s0 = (k // 16) * 16 * D
                                nn = (k % 16 + 1) * D
                                nc.any.tensor_copy(
                                    staging[:, s0 : s0 + nn], p2[:, :nn]
                                )

            nc.sync.dma_start(out=out_d[:, :], in_=staging[:, :])

    _BASS_PROG = nc
    return nc


def _compute_bass(x, x_0, edge_index, weight1):
    import time as _time

    from concourse.bass_utils import run_bass_kernel_spmd

    _t0 = _time.perf_counter()
    x_bf, x0fm, src_dev, dr_dev, w_f32, spill = _preprocess(
        x, x_0, edge_index, weight1
    )
    _t1 = _time.perf_counter()
    nc = _build_bass_program()
    _t2 = _time.perf_counter()

    in_maps = [
        {
            "x": x_bf,
            "x0": x0fm[c],
            "src": src_dev[c],
            "dr": dr_dev[c],
            "w": w_f32,
        }
        for c in range(NCORES)
    ]
    res = run_bass_kernel_spmd(nc, in_maps, list(range(NCORES))).results
    _t3 = _time.perf_counter()
    print(
        f"bass stages: preprocess {_t1-_t0:.2f}s build {_t2-_t1:.2f}s "
        f"compile+run {_t3-_t2:.2f}s"
    )

    out = np.empty((N, D), dtype=np.float32)
    for c in range(NCORES):
        o = np.asarray(res[c]["out"], dtype=np.float32)
        o = o.reshape(128, NWIN, D).transpose(1, 0, 2).reshape(NODES_PAD, D)
        out[c * NLOC : (c + 1) * NLOC] = o[:NLOC]

    if spill is not None:
        ss, ds = spill
        m_np = (1.0 - BETA) * np.eye(D, dtype=np.float32) + BETA * np.asarray(
            weight1, dtype=np.float32
        )
        contrib = (1.0 - ALPHA) * np.asarray(x, dtype=np.float32)[ss] @ m_np
        np.add.at(out, ds, contrib)
    return out


def _compute_numpy(x, x_0, edge_index, weight1):
    src = np.asarray(edge_index[0], dtype=np.int64)
    dst = np.asarray(edge_index[1], dtype=np.int64)
    x = np.asarray(x, dtype=np.float32)
    x_0 = np.asarray(x_0, dtype=np.float32)
    weight1 = np.asarray(weight1, dtype=np.float32)

    gathered = x[src]  # [E, D]
    agg = np.empty((N, D), dtype=np.float32)
    for d in range(D):
        agg[:, d] = np.bincount(dst, weights=gathered[:, d], minlength=N)

    out = (1.0 - ALPHA) * agg + ALPHA * x_0
    out = (1.0 - BETA) * out + BETA * (out @ weight1)
    return out.astype(np.float32)


def _compute_jax_neuron_local(x, x_0, edge_index, weight1):
    """Fallback: JAX shard_map on the neuron cores (slow to compile)."""
    import jax
    import jax.numpy as jnp
    from jax.sharding import Mesh, PartitionSpec as P
    from jax.experimental.shard_map import shard_map

    devs = jax.devices()
    if len(devs) < NCORES:
        raise RuntimeError("need 8 cores")
    mesh = Mesh(np.array(devs[:NCORES]), ("i",))

    n_loc = NLOC
    src = np.asarray(edge_index[0], dtype=np.int32)
    dst = np.asarray(edge_index[1], dtype=np.int32)
    bucket = dst // n_loc
    order = np.argsort(bucket, kind="stable")
    src_s, dst_s = src[order], dst[order]
    counts = np.bincount(bucket, minlength=NCORES)
    cap = int(counts.max())
    src_p = np.zeros((NCORES, cap), dtype=np.int32)
    dst_p = np.full((NCORES, cap), n_loc, dtype=np.int32)
    off = 0
    for c in range(NCORES):
        cnt = int(counts[c])
        src_p[c, :cnt] = src_s[off : off + cnt]
        dst_p[c, :cnt] = dst_s[off : off + cnt] - c * n_loc
        off += cnt

    xj = jnp.asarray(x, dtype=jnp.float32)
    x0j = jnp.asarray(x_0, dtype=jnp.float32)
    wj = jnp.asarray(weight1, dtype=jnp.float32)
    srcj = jnp.asarray(src_p)
    dstj = jnp.asarray(dst_p)

    def shard_fn(src_l, dst_l, x_full, x0_l, w):
        gathered = x_full[src_l[0]]
        agg_l = jax.ops.segment_sum(gathered, dst_l[0], num_segments=n_loc)
        out = (1.0 - ALPHA) * agg_l + ALPHA * x0_l
        out = (1.0 - BETA) * out + BETA * (out @ w)
        return out

    fn = jax.jit(
        shard_map(
            shard_fn,
            mesh=mesh,
            in_specs=(P("i"), P("i"), P(), P("i"), P()),
            out_specs=P("i"),
        )
    )
    out = fn(srcj, dstj, xj, x0j, wj)
    return np.asarray(jax.device_get(out), dtype=np.float32)


def kernel(x, x_0, edge_index, weight1):
    try:
        return _compute_bass(x, x_0, edge_index, weight1)
    except Exception:
        import traceback

        traceback.print_exc()
    try:
        return _compute_jax_neuron_local(x, x_0, edge_index, weight1)
    except Exception:
        return _compute_numpy(x, x_0, edge_index, weight1)


# revision 19
# speedup vs baseline: 2.3994x; 1.3218x over previous
import math

import numpy as np

# GCNII layer constants (match the reference problem definition).
N = 100000
D = 32
NCORES = 8
NLOC = N // NCORES  # 12500
ALPHA = 0.1
THETA = 0.5
LAYER = 8
BETA = math.log(THETA / (LAYER + 1) + 1.0)

# Device tiling constants.
WIN = 128            # nodes per window (= one-hot matmul width)
NWIN = 98            # windows per core (97*128 + 84 = 12500)
NB = 4               # source blocks (dma_gather indices are int16)
BLK = N // NB        # 25000 rows per block
BT = 5               # tiles (128 edge slots) per (window, block)
TPW = NB * BT        # 20 tiles per window
EW = 128             # padded x row width in bf16 (= 256B gather element)
CHW = 7              # windows per gather chunk
NCH = NWIN // CHW    # 14 chunks per core
TILES_CH = CHW * TPW         # 140 tiles per chunk
ROWS_CALL = CHW * BT * 128   # 4480 rows per dma_gather call
NODES_PAD = NWIN * WIN       # 12544

_BASS_PROG = None  # cached across calls in one process


def _np_bf16():
    import ml_dtypes

    return ml_dtypes.bfloat16


def _preprocess(x, x_0, edge_index, weight1):
    """Sort edges by destination, bucket into per-core / per-window /
    per-source-block padded slots, and build the device-side layouts."""
    bf16 = _np_bf16()
    src = np.ascontiguousarray(edge_index[0], dtype=np.int32)
    dst = np.ascontiguousarray(edge_index[1], dtype=np.int32)
    E = src.shape[0]

    blk = src // BLK
    core = dst // NLOC
    loc = dst - core * NLOC
    wl = loc // WIN
    gkey = (core * NWIN + wl) * NB + blk

    order = np.argsort(gkey, kind="stable")
    gk = gkey[order]
    ss = src[order]
    ds = dst[order]

    counts = np.bincount(gk, minlength=NCORES * NWIN * NB)
    starts = np.concatenate([[0], np.cumsum(counts)[:-1]])
    rank = np.arange(E, dtype=np.int64) - starts[gk]
    keep = rank < BT * 128

    core_o = gk // (NWIN * NB)
    wl_o = (gk // NB) % NWIN
    blk_o = gk % NB
    ch = wl_o // CHW
    wp = wl_o - ch * CHW
    t = rank // 128
    lane = rank % 128
    tile_in_chunk = blk_o * (CHW * BT) + wp * BT + t

    # dr: [core, ch, lane, tile_in_chunk] bf16, pads -1
    pos_dr = ((core_o * NCH + ch) * 128 + lane) * TILES_CH + tile_in_chunk
    dr_dev = np.full(NCORES * NCH * 128 * TILES_CH, -1.0, dtype=np.float32)
    dstrel = (loc - wl * WIN)[order]
    dr_dev[pos_dr[keep]] = dstrel[keep]
    dr_dev = dr_dev.astype(bf16).reshape(NCORES, NCH, 128, 1, NB, CHW * BT)

    # idx: per (core, ch, blk) gather call, row i = (wp*BT+t)*128 + lane,
    # wrapped: element i at [i % 16, i // 16], replicated over 8 groups.
    i_in_call = (wp * BT + t) * 128 + lane
    pos_ix = ((core_o * NCH + ch) * NB + blk_o) * ROWS_CALL + i_in_call
    idxflat = np.zeros(NCORES * NCH * NB * ROWS_CALL, dtype=np.int16)
    idxflat[pos_ix[keep]] = (ss - blk_o * BLK)[keep].astype(np.int16)
    idxflat = idxflat.reshape(NCORES, NCH, NB, ROWS_CALL // 16, 16)
    idx_dev = np.ascontiguousarray(np.swapaxes(idxflat, 3, 4))  # [.., 16, R/16]
    idx_dev = np.tile(idx_dev, (1, 1, 1, 8, 1))  # [NCORES, NCH, NB, 128, R/16]

    x_bf = np.ascontiguousarray(x, dtype=np.float32).astype(bf16)

    # x0 feature-major per core, padded to NODES_PAD nodes.
    x0T = np.ascontiguousarray(np.asarray(x_0, dtype=np.float32).T)  # [32, N]
    x0fm = np.zeros((NCORES, D, NODES_PAD), dtype=np.float32)
    for c in range(NCORES):
        x0fm[c, :, :NLOC] = x0T[:, c * NLOC : (c + 1) * NLOC]
    x0fm = x0fm.astype(bf16)

    w_f32 = np.ascontiguousarray(weight1, dtype=np.float32)

    spill = None
    if not np.all(keep):
        sp = ~keep
        spill = (ss[sp], ds[sp])
    return x_bf, x0fm, idx_dev, dr_dev, w_f32, spill


def _build_bass_program():
    global _BASS_PROG
    if _BASS_PROG is not None:
        return _BASS_PROG

    import concourse.mybir as mybir
    from concourse import bacc
    from concourse.masks import make_identity
    from concourse.tile import TileContext

    dt = mybir.dt
    op = mybir.AluOpType

    # Bacc (not plain Bass): its lowering pipeline splits multi-sem waits
    # into event-semaphore chains; walrus rejects raw instructions with
    # more than one sync wait.
    nc = bacc.Bacc(None)
    x_d = nc.declare_dram_parameter("x", [N, D], dt.bfloat16, False)
    x0_d = nc.declare_dram_parameter("x0", [D, NODES_PAD], dt.bfloat16, False)
    idx_d = nc.declare_dram_parameter(
        "idx", [NCH, NB, 128, ROWS_CALL // 16], dt.int16, False
    )
    dr_d = nc.declare_dram_parameter(
        "dr", [NCH, 128, 1, NB, CHW * BT], dt.bfloat16, False
    )
    w_d = nc.declare_dram_parameter("w", [D, D], dt.float32, False)
    out_d = nc.declare_dram_parameter("out", [128, NWIN * D], dt.float32, True)

    x_pad = nc.dram_tensor("x_pad", [N, EW], dt.bfloat16)

    with TileContext(nc) as tc:
        with (
            tc.tile_pool(name="const", bufs=1) as constp,
            tc.tile_pool(name="idx", bufs=2) as idxp,
            tc.tile_pool(name="drt", bufs=2) as drp,
            tc.tile_pool(name="gat", bufs=2) as gp,
            tc.tile_pool(name="sel", bufs=4) as sp_,
            tc.tile_pool(name="agg", bufs=3) as abp,
            tc.tile_pool(name="pagg", bufs=4, space="PSUM") as pap,
            tc.tile_pool(name="pout", bufs=2, space="PSUM") as p2p,
        ):
            # Pad x rows 32 -> 128 bf16 in DRAM so each gather element is
            # the required 256 bytes.
            nc.sync.dma_start(out=x_pad[:, 0:D], in_=x_d[:, :])

            # iota [128, WIN, NB, BT] (value = window-node index j); every
            # operand of the one-hot compare is stride-1 in its last axis
            # -> DVE 2x mode.
            iota_i = constp.tile([128, WIN, NB, BT], dt.int16)
            nc.gpsimd.iota(
                iota_i[:, :, :, :], pattern=[[1, WIN], [0, NB], [0, BT]],
                base=0, channel_multiplier=0,
            )
            iota_bf = constp.tile([128, WIN, NB, BT], dt.bfloat16)
            nc.vector.tensor_copy(iota_bf[:, :, :, :], iota_i[:, :, :, :])

            # GCNII combination matrices Ma = 0.9((1-b)I + bW), Mb = Ma/9.
            w_sb = constp.tile([D, D], dt.float32)
            nc.sync.dma_start(out=w_sb[:, :], in_=w_d[:, :])
            eye = constp.tile([D, D], dt.float32)
            make_identity(nc, eye[:, :])
            wa = constp.tile([D, D], dt.float32)
            nc.vector.tensor_scalar_mul(wa[:, :], w_sb[:, :], 0.9 * BETA)
            wb = constp.tile([D, D], dt.float32)
            nc.vector.tensor_scalar_mul(wb[:, :], w_sb[:, :], 0.1 * BETA)
            eya = constp.tile([D, D], dt.float32)
            nc.vector.tensor_scalar_mul(eya[:, :], eye[:, :], 0.9 * (1.0 - BETA))
            eyb = constp.tile([D, D], dt.float32)
            nc.vector.tensor_scalar_mul(eyb[:, :], eye[:, :], 0.1 * (1.0 - BETA))
            ma = constp.tile([D, D], dt.bfloat16)
            nc.vector.tensor_tensor(out=ma[:, :], in0=eya[:, :], in1=wa[:, :], op=op.add)
            mb = constp.tile([D, D], dt.bfloat16)
            nc.vector.tensor_tensor(out=mb[:, :], in0=eyb[:, :], in1=wb[:, :], op=op.add)

            x0_sb = constp.tile([D, NODES_PAD], dt.bfloat16)
            nc.sync.dma_start(out=x0_sb[:, :], in_=x0_d[:, :])

            staging = constp.tile([128, NWIN * D], dt.float32)

            pa = None
            p2 = None
            for chi in range(NCH):
                idx_t = idxp.tile([128, NB, ROWS_CALL // 16], dt.int16)
                nc.sync.dma_start(
                    out=idx_t[:, :, :], in_=idx_d[chi].transpose([1, 0, 2])
                )
                dr_t = drp.tile([128, 1, NB, CHW * BT], dt.bfloat16)
                nc.scalar.dma_start(out=dr_t[:, :, :, :], in_=dr_d[chi, :, :, :, :])

                g_t = gp.tile([128, TILES_CH, EW], dt.bfloat16)
                for b in range(NB):
                    nc.gpsimd.dma_gather(
                        out_ap=g_t[:, b * (CHW * BT) : (b + 1) * (CHW * BT), :],
                        in_ap=x_pad[b * BLK : (b + 1) * BLK, :],
                        idxs_ap=idx_t[:, b, :],
                        num_idxs=ROWS_CALL,
                        num_idxs_reg=ROWS_CALL,
                        elem_size=EW,
                    )

                for m in range(CHW):
                    w = chi * CHW + m
                    s_t = sp_.tile([128, WIN, NB, BT], dt.bfloat16)
                    nc.vector.tensor_tensor(
                        out=s_t[:, :, :, :],
                        in0=iota_bf[:, :, :, :],
                        in1=dr_t[:, :, :, m * BT : (m + 1) * BT].broadcast_to(
                            [128, WIN, NB, BT]
                        ),
                        op=op.is_equal,
                    )
                    if w % 4 == 0:
                        pa = pap.tile([D, 4 * WIN], dt.float32)
                    for b in range(NB):
                        for t in range(BT):
                            nc.tensor.matmul(
                                out=pa[:, (w % 4) * WIN : (w % 4 + 1) * WIN],
                                lhsT=g_t[:, b * (CHW * BT) + m * BT + t, 0:D],
                                rhs=s_t[:, :, b, t],
                                start=(b == 0 and t == 0),
                                stop=(b == NB - 1 and t == BT - 1),
                            )
                    if w % 4 == 3 or w == NWIN - 1:
                        g0 = (w // 4) * 4
                        ncols = (w - g0 + 1) * WIN
                        ab = abp.tile([D, 4 * WIN], dt.bfloat16)
                        # DVE (not nc.any->ACT): the psum slot release must
                        # stay on the clock the S-matmuls already wait on.
                        nc.vector.tensor_copy(ab[:, :ncols], pa[:, :ncols])
                        for k in range(g0, w + 1):
                            if k % 16 == 0:
                                p2 = p2p.tile([128, 16 * D], dt.float32)
                            c0 = (k % 16) * D
                            nc.tensor.matmul(
                                out=p2[:, c0 : c0 + D],
                                lhsT=ab[:, (k - g0) * WIN : (k - g0 + 1) * WIN],
                                rhs=ma[:, :],
                                start=True,
                                stop=False,
                            )
                            nc.tensor.matmul(
                                out=p2[:, c0 : c0 + D],
                                lhsT=x0_sb[:, k * WIN : (k + 1) * WIN],
                                rhs=mb[:, :],
                                start=False,
                                stop=True,
                            )
                            if k % 16 == 15 or k == NWIN - 1:
                                s0 = (k // 16) * 16 * D
                                nn = (k % 16 + 1) * D
                                nc.any.tensor_copy(
                                    staging[:, s0 : s0 + nn], p2[:, :nn]
                                )

            nc.sync.dma_start(out=out_d[:, :], in_=staging[:, :])

    nc.finalize()
    _BASS_PROG = nc
    return nc


def _compute_bass(x, x_0, edge_index, weight1):
    import time as _time

    from concourse.bass_utils import run_bass_kernel_spmd

    _t0 = _time.perf_counter()
    x_bf, x0fm, idx_dev, dr_dev, w_f32, spill = _preprocess(
        x, x_0, edge_index, weight1
    )
    _t1 = _time.perf_counter()
    nc = _build_bass_program()
    _t2 = _time.perf_counter()

    in_maps = [
        {
            "x": x_bf,
            "x0": x0fm[c],
            "idx": idx_dev[c],
            "dr": dr_dev[c],
            "w": w_f32,
        }
        for c in range(NCORES)
    ]
    res = run_bass_kernel_spmd(nc, in_maps, list(range(NCORES))).results
    _t3 = _time.perf_counter()
    print(
        f"bass stages: preprocess {_t1-_t0:.2f}s build {_t2-_t1:.2f}s "
        f"compile+run {_t3-_t2:.2f}s"
    )

    out = np.empty((N, D), dtype=np.float32)
    for c in range(NCORES):
        o = np.asarray(res[c]["out"], dtype=np.float32)
        o = o.reshape(128, NWIN, D).transpose(1, 0, 2).reshape(NODES_PAD, D)
        out[c * NLOC : (c + 1) * NLOC] = o[:NLOC]

    if spill is not None:
        ss, ds = spill
        m_np = (1.0 - BETA) * np.eye(D, dtype=np.float32) + BETA * np.asarray(
            weight1, dtype=np.float32
        )
        contrib = (1.0 - ALPHA) * np.asarray(x, dtype=np.float32)[ss] @ m_np
        np.add.at(out, ds, contrib)
    return out


def _compute_numpy(x, x_0, edge_index, weight1):
    src = np.asarray(edge_index[0], dtype=np.int64)
    dst = np.asarray(edge_index[1], dtype=np.int64)
    x = np.asarray(x, dtype=np.float32)
    x_0 = np.asarray(x_0, dtype=np.float32)
    weight1 = np.asarray(weight1, dtype=np.float32)

    gathered = x[src]  # [E, D]
    agg = np.empty((N, D), dtype=np.float32)
    for d in range(D):
        agg[:, d] = np.bincount(dst, weights=gathered[:, d], minlength=N)

    out = (1.0 - ALPHA) * agg + ALPHA * x_0
    out = (1.0 - BETA) * out + BETA * (out @ weight1)
    return out.astype(np.float32)


def _compute_jax_neuron_local(x, x_0, edge_index, weight1):
    """Fallback: JAX shard_map on the neuron cores (slow to compile)."""
    import jax
    import jax.numpy as jnp
    from jax.sharding import Mesh, PartitionSpec as P
    from jax.experimental.shard_map import shard_map

    devs = jax.devices()
    if len(devs) < NCORES:
        raise RuntimeError("need 8 cores")
    mesh = Mesh(np.array(devs[:NCORES]), ("i",))

    n_loc = NLOC
    src = np.asarray(edge_index[0], dtype=np.int32)
    dst = np.asarray(edge_index[1], dtype=np.int32)
    bucket = dst // n_loc
    order = np.argsort(bucket, kind="stable")
    src_s, dst_s = src[order], dst[order]
    counts = np.bincount(bucket, minlength=NCORES)
    cap = int(counts.max())
    src_p = np.zeros((NCORES, cap), dtype=np.int32)
    dst_p = np.full((NCORES, cap), n_loc, dtype=np.int32)
    off = 0
    for c in range(NCORES):
        cnt = int(counts[c])
        src_p[c, :cnt] = src_s[off : off + cnt]
        dst_p[c, :cnt] = dst_s[off : off + cnt] - c * n_loc
        off += cnt

    xj = jnp.asarray(x, dtype=jnp.float32)
    x0j = jnp.asarray(x_0, dtype=jnp.float32)
    wj = jnp.asarray(weight1, dtype=jnp.float32)
    srcj = jnp.asarray(src_p)
    dstj = jnp.asarray(dst_p)

    def shard_fn(src_l, dst_l, x_full, x0_l, w):
        gathered = x_full[src_l[0]]
        agg_l = jax.ops.segment_sum(gathered, dst_l[0], num_segments=n_loc)
        out = (1.0 - ALPHA) * agg_l + ALPHA * x0_l
        out = (1.0 - BETA) * out + BETA * (out @ w)
        return out

    fn = jax.jit(
        shard_map(
            shard_fn,
            mesh=mesh,
            in_specs=(P("i"), P("i"), P(), P("i"), P()),
            out_specs=P("i"),
        )
    )
    out = fn(srcj, dstj, xj, x0j, wj)
    return np.asarray(jax.device_get(out), dtype=np.float32)


def kernel(x, x_0, edge_index, weight1):
    try:
        return _compute_bass(x, x_0, edge_index, weight1)
    except Exception:
        import traceback

        traceback.print_exc()
    try:
        return _compute_jax_neuron_local(x, x_0, edge_index, weight1)
    except Exception:
        return _compute_numpy(x, x_0, edge_index, weight1)
